# revision 2
# baseline (speedup 1.0000x reference)
# Deformable Conv2d (B=4, C=CO=64, H=W=192, K=3, pad=1) on 8 Trainium2 cores.
#
# v2: multi-engine combine.  Response planes r_{k,s} = W_k @ x(shifted) are
# computed by merged-M (128-row) matmuls into packed PSUM "bins" (<=512 cols).
# Bins are evicted to SBUF (bf16) by ACT (most) + DVE (some); the bilinear-hat
# modulated accumulation out[px,:] += wyx_t(px) * r_t[px,:] then runs as
# scalar_tensor_tensor ops split across DVE and GPSIMD reading SBUF (127ns vs
# 352ns for PSUM-sourced ops).  Hat weights are built on DVE (abs_max/max ALU
# ops), tap products on GPSIMD.  The offset conv keeps its own PSUM tile.
import os
import numpy as np

B, C, CO, H, W = 4, 64, 64, 192, 192
K, PAD, KK = 3, 1, 9
N_CORES = 8
HALVES = N_CORES // B
ROWS = H // HALVES               # 96 rows per core
HALO = 3
PADC = 3
WP = W + 2 * PADC                # 198
RSLAB = ROWS + 2 * HALO          # 102
LOOPT = int(os.environ.get("DFC_LOOPT", "1"))
CB = 3
TILE_ROWS = ROWS // 2            # 48 row-pairs
N_TILES = TILE_ROWS * CB         # 144

DYPERM = [0, 4, 8, 12, 16, 3, 7, 11, 15]
DXPERM = [2, 6, 10, 14, 1, 5, 9, 13, 17]
BASE = [(k // 3 - 1, k % 3 - 1) for k in range(KK)]

# shift groups: (sy, sx, ks); center split to keep nk <= 8
SHIFTS = []
for sy in range(-2, 3):
    for sx in range(-2, 3):
        ks = [k for k in range(KK)
              if abs(sy - BASE[k][0]) <= 1 and abs(sx - BASE[k][1]) <= 1]
        if ks:
            SHIFTS.append((sy, sx, ks))
GROUPS = []
for sy, sx, ks in SHIFTS:
    for i in range(0, len(ks), 8):
        GROUPS.append((sy, sx, ks[i:i + 8]))

# bin-pack groups into PSUM tiles of <=8 k-slots (512 cols)
BINS = []            # list of list of group-indices
_order = sorted(range(len(GROUPS)), key=lambda g: -len(GROUPS[g][2]))
for g in _order:
    nk = len(GROUPS[g][2])
    for b in BINS:
        if b[0] + nk <= 8:
            b[0] += nk
            b[1].append(g)
            break
    else:
        BINS.append([nk, [g]])
BINS = [b[1] for b in BINS]
N_BINS = len(BINS)

# wstack column order: bins in order, groups within bin, k within group
WCOLS = []           # (group_idx, k) in wstack col order
BIN_LAYOUT = []      # per bin: (ngroup list with (g, col_off))
for bi, b in enumerate(BINS):
    off = 0
    lay = []
    for g in b:
        lay.append((g, off))
        for k in GROUPS[g][2]:
            WCOLS.append((g, k))
        off += len(GROUPS[g][2]) * CO
    BIN_LAYOUT.append(lay)
WTOT = len(WCOLS) * CO           # 81*64

# combine term schedule: term t = (bin, slot) in eviction order.
# Round-robin DVE/GPS with ratio ~ 51:30.
TERMS = []
for bi, b in enumerate(BINS):
    slot = 0
    for g in b:
        sy, sx, ks = GROUPS[g]
        for k in ks:
            ui = sy - BASE[k][0] + 1
            vi = sx - BASE[k][1] + 1
            TERMS.append((bi, slot, k, ui, vi))
            slot += 1

N_GPS = int(os.environ.get("DFC_NGPS", "22"))
N_ACT_EVICT = int(os.environ.get("DFC_NACTEV", "9"))   # bins evicted by ACT
_CACHE = {}


def _build_program():
    import concourse.bacc as bacc
    import concourse.mybir as mybir
    from concourse import tile

    f32 = mybir.dt.float32
    bf16 = mybir.dt.bfloat16
    MUL = mybir.AluOpType.mult
    ADD = mybir.AluOpType.add
    SUB = mybir.AluOpType.subtract
    ABSMAX = mybir.AluOpType.abs_max
    MAX = mybir.AluOpType.max
    AF = mybir.ActivationFunctionType

    nc = bacc.Bacc("TRN2", num_devices=N_CORES)
    xslab_d = nc.dram_tensor("xslab", [C + 1, RSLAB, WP], bf16, kind="ExternalInput")
    woffb_d = nc.dram_tensor("woffb", [C + 1, KK * 2 * KK], bf16, kind="ExternalInput")
    wstack_d = nc.dram_tensor("wstack", [C, WTOT], bf16, kind="ExternalInput")
    out_d = nc.dram_tensor("out", [N_TILES * 128, CO], f32, kind="ExternalOutput")

    # GPS/DVE term split: exactly N_GPS terms go to GPSIMD, spread evenly
    NT = len(TERMS)
    gps_set = {t for t in range(NT)
               if (t * N_GPS) // NT != ((t - 1) * N_GPS) // NT}

    with tile.TileContext(nc) as tc:
        with (
            tc.tile_pool(name="slab", bufs=1) as slab_pool,
            tc.tile_pool(name="consts", bufs=1) as const_pool,
            tc.tile_pool(name="wts", bufs=2) as wts_pool,
            tc.tile_pool(name="rb", bufs=1) as rb_pool,
            tc.tile_pool(name="acc", bufs=2) as acc_pool,
            tc.tile_pool(name="psum", bufs=3, space="PSUM") as psum_pool,
            tc.tile_pool(name="ppsum", bufs=2, space="PSUM") as ppsum_pool,
        ):
            xsb = slab_pool.tile([C + 1, RSLAB, WP], bf16)
            nc.sync.dma_start(xsb[:, :, :], xslab_d.ap()[:, :, :])
            woffb = const_pool.tile([C + 1, KK * 2 * KK], bf16)
            nc.sync.dma_start(woffb[:, :], woffb_d.ap()[:, :])
            wstack = const_pool.tile([C, WTOT], bf16)
            nc.sync.dma_start(wstack[:, :], wstack_d.ap()[:, :])

            import contextlib
            loop_cm = tc.For_i(0, LOOPT, 1) if LOOPT > 1 else contextlib.nullcontext()
            with loop_cm:
              for hh in range(TILE_ROWS):
                r0 = 2 * hh + HALO

                # ---- offset conv (merged-M): p_ps [128, 54] ----
                p_ps = ppsum_pool.tile([128, CB * 2 * KK], f32, tag="p")
                for cb in range(CB):
                    c0 = PADC + cb * 64
                    for row in range(2):
                        for k in range(KK):
                            by, bx = BASE[k]
                            nc.tensor.matmul(
                                p_ps[row * 64:(row + 1) * 64,
                                     cb * 18:(cb + 1) * 18],
                                xsb[0:C + 1, r0 + row + by,
                                    c0 + bx:c0 + bx + 64],
                                woffb[:, k * 18:(k + 1) * 18],
                                start=(k == 0), stop=(k == KK - 1),
                            )

                # ---- hat weights on DVE ----
                # p layout: [128, (cb,18)]; dy cols = cb*18 + 0..8, dx + 9..17
                wyp = {}
                for ax in range(2):
                    d = p_ps.rearrange("p (c t) -> p c t", t=2 * KK)[
                        :, :, ax * KK:(ax + 1) * KK]   # [128, 3, 9] strided
                    a = wts_pool.tile([128, CB * KK], f32, tag=f"a{ax}")
                    w0 = wts_pool.tile([128, CB * KK], f32, tag=f"w0{ax}")
                    wp = wts_pool.tile([128, CB * KK], f32, tag=f"wp{ax}")
                    wm = wts_pool.tile([128, CB * KK], f32, tag=f"wm{ax}")
                    # wp = max(d, 0)
                    nc.vector.tensor_scalar(
                        wp.rearrange("p (c t) -> p c t", t=KK)[:, :, :],
                        d, 0.0, None, MAX)
                    # wm = max(-d, 0)
                    nc.vector.tensor_scalar(
                        wm.rearrange("p (c t) -> p c t", t=KK)[:, :, :],
                        d, -1.0, 0.0, MUL, MAX)
                    # a = wp + wm = |d|
                    nc.vector.tensor_tensor(a[:, :], wp[:, :], wm[:, :], ADD)
                    # w0 = 1 - a
                    nc.vector.tensor_scalar(w0[:, :], a[:, :], -1.0, 1.0, MUL, ADD)
                    wyp[ax] = [wm, w0, wp]

                # ---- products on GPSIMD: wyx[(uv), (cb,k)] ----
                wyx = wts_pool.tile([128, 9 * CB * KK], f32, tag="wyx")
                for ui in range(3):
                    for vi in range(3):
                        nc.gpsimd.tensor_tensor(
                            wyx[:, (ui * 3 + vi) * 27:(ui * 3 + vi + 1) * 27],
                            wyp[0][ui][:, :], wyp[1][vi][:, :], MUL)

                for cb in range(CB):
                    t_idx = hh * CB + cb
                    c0 = PADC + cb * 64

                    accs = [acc_pool.tile([128, CO], f32, tag=f"acc{a}",
                                          name=f"acc{a}_{t_idx}")
                            for a in range(2)]
                    gaccs = [acc_pool.tile([128, CO], f32, tag=f"gacc{a}",
                                           name=f"gacc{a}_{t_idx}")
                             for a in range(2)]
                    started = [False] * 2
                    gstarted = [False] * 2

                    term_i = 0
                    di = 0
                    gi = 0
                    woff = 0
                    for pi in range(0, len(BIN_LAYOUT), 2):
                        pair = BIN_LAYOUT[pi:pi + 2]
                        cols0 = sum(len(GROUPS[g][2]) for g, _ in pair[0]) * CO
                        cols = cols0
                        if len(pair) > 1:
                            cols += sum(len(GROUPS[g][2]) for g, _ in pair[1]) * CO
                        bin_ps = psum_pool.tile([128, 1024], f32,
                                                tag="bin",
                                                name=f"bin{pi}_{t_idx}")
                        base = 0
                        for lay in pair:
                            for g, coff in lay:
                                sy, sx, ks = GROUPS[g]
                                nk = len(ks)
                                for row in range(2):
                                    nc.tensor.matmul(
                                        bin_ps[row * 64:(row + 1) * 64,
                                               base + coff:base + coff + nk * CO],
                                        xsb[0:C, r0 + row + sy,
                                            c0 + sx:c0 + sx + 64],
                                        wstack[:, woff:woff + nk * CO],
                                        start=True, stop=True)
                                woff += nk * CO
                            base = 512
                        # evict pair -> SBUF bf16 (base-512 gap handled via 2D AP)
                        rb = rb_pool.tile([128, 1024], bf16, tag="rb",
                                          name=f"rb{pi}_{t_idx}", bufs=4)
                        nc.scalar.activation(rb[:, 0:cols0], bin_ps[:, 0:cols0],
                                             AF.Copy)
                        if cols > cols0:
                            nc.scalar.activation(rb[:, 512:512 + cols - cols0],
                                                 bin_ps[:, 512:512 + cols - cols0],
                                                 AF.Copy)
                        # combine terms of this pair
                        for slot in range(cols // CO):
                            if slot * CO >= cols0 and slot * CO < 512:
                                continue
                            _, _, k, ui, vi = TERMS[term_i]
                            col = (ui * 3 + vi) * 27 + cb * KK + k
                            sc = wyx[:, col:col + 1]
                            rsl = rb[:, slot * CO:(slot + 1) * CO]
                            if term_i in gps_set:
                                a = gi % 2
                                gi += 1
                                if not gstarted[a]:
                                    nc.gpsimd.tensor_scalar(
                                        gaccs[a][:, :], rsl, sc, None, MUL)
                                    gstarted[a] = True
                                else:
                                    gt = acc_pool.tile([128, CO], f32,
                                                       tag=f"gtmp{a}",
                                                       name=f"gt{a}_{t_idx}_{term_i}")
                                    nc.gpsimd.tensor_scalar(
                                        gt[:, :], rsl, sc, None, MUL)
                                    nc.gpsimd.tensor_tensor(
                                        gaccs[a][:, :], gaccs[a][:, :],
                                        gt[:, :], ADD)
                            else:
                                a = di % 2
                                di += 1
                                if not started[a]:
                                    nc.vector.tensor_scalar(
                                        accs[a][:, :], rsl, sc, None, MUL)
                                    started[a] = True
                                else:
                                    nc.vector.scalar_tensor_tensor(
                                        accs[a][:, :], rsl, sc, accs[a][:, :],
                                        MUL, ADD)
                            term_i += 1

                    # finals
                    nc.gpsimd.tensor_tensor(gaccs[0][:, :], gaccs[0][:, :],
                                            gaccs[1][:, :], ADD)
                    nc.vector.tensor_tensor(accs[0][:, :], accs[0][:, :],
                                            accs[1][:, :], ADD)
                    nc.vector.tensor_tensor(accs[0][:, :], accs[0][:, :],
                                            gaccs[0][:, :], ADD)
                    nc.sync.dma_start(
                        out_d.ap()[t_idx * 128:(t_idx + 1) * 128, :],
                        accs[0][:, :])

    nc.compile()
    return nc


def _prep_weights(w_deform, w_offset, b_offset):
    perm = DYPERM + DXPERM
    wo = w_offset[perm]
    bo = b_offset[perm]
    woffb = np.zeros((C + 1, KK * 18), np.float32)
    for k in range(KK):
        ky, kx = k // 3, k % 3
        woffb[:C, k * 18:(k + 1) * 18] = wo[:, :, ky, kx].T
    woffb[C, 4 * 18:5 * 18] = bo
    blocks = []
    for g, k in WCOLS:
        blocks.append(w_deform[:, :, k // 3, k % 3].T)   # [C, CO]
    wstack = np.concatenate(blocks, axis=1).astype(np.float32)
    import ml_dtypes
    return woffb.astype(ml_dtypes.bfloat16), wstack.astype(ml_dtypes.bfloat16)


def kernel(x, w_deform, w_offset, b_offset):
    from concourse.bass_utils import run_bass_kernel_spmd

    if "nc" not in _CACHE:
        _CACHE["nc"] = _build_program()
    nc = _CACHE["nc"]

    woffb, wstack = _prep_weights(
        np.asarray(w_deform, np.float32),
        np.asarray(w_offset, np.float32),
        np.asarray(b_offset, np.float32))

    x = np.asarray(x, np.float32)
    in_maps = []
    for core in range(N_CORES):
        b, half = core // HALVES, core % HALVES
        import ml_dtypes
        slab = np.zeros((C + 1, RSLAB, WP), ml_dtypes.bfloat16)
        slab[C] = 1.0
        r_lo = half * ROWS - HALO
        r_hi = half * ROWS + ROWS + HALO
        src_lo, src_hi = max(r_lo, 0), min(r_hi, H)
        slab[:C, src_lo - r_lo:src_hi - r_lo, PADC:PADC + W] = \
            x[b, :, src_lo:src_hi, :].astype(ml_dtypes.bfloat16)
        in_maps.append({"xslab": slab, "woffb": woffb, "wstack": wstack})

    res = run_bass_kernel_spmd(nc, in_maps, core_ids=list(range(N_CORES)))

    out = np.empty((B, CO, H, W), np.float32)
    for core in range(N_CORES):
        b, half = core // HALVES, core % HALVES
        o = res.results[core]["out"]
        o = o.reshape(TILE_ROWS, CB, 2, 64, CO)
        o = o.transpose(4, 0, 2, 1, 3).reshape(CO, ROWS, W)
        out[b, :, half * ROWS:(half + 1) * ROWS, :] = o
    return out


if __name__ == "__main__":
    xs = {k: np.load(f"/tmp/in_{k}.npy") for k in
          ("x", "w_deform", "w_offset", "b_offset")}
    got = kernel(**xs)
    exp = np.load("/tmp/expected.npy")
    err = np.abs(got - exp)
    rel = np.linalg.norm(got - exp) / np.linalg.norm(exp)
    print(f"absmax={err.max():.6f} rel-l2={rel:.3e}")
